# revision 22
# baseline (speedup 1.0000x reference)
"""Trainium2 Bass kernel for nn_DeepSetAttentionModel (segment_reduce).

Algebraic simplification: the psi-MLP / segment-mean branch adds
`agg[seg] @ W_k[48:]` to every key in a segment — a per-segment constant per
head in `preattn`.  Segment softmax is invariant to per-segment constants, so
the entire psi branch cancels from the output and is dropped.  What remains of
the attention logits is `z = x @ M1` with
`M1 = (W_k[:48].reshape(48,H,D) . W_q) / sqrt(D)` (folded on host, O(weights)).

Sharding: data-parallel across patients — 8 whole segments per core, weights
replicated.  Each segment occupies 4608 feature-major columns of
x_T [48, 8*4608] (bf16): cols 0..4095 time rows, col 4096 the demo-encoder
row, cols 4097..4607 zero pad (their z is forced to -1e30 so softmax gives
them exactly 0 weight and phi(0)=0 contributes nothing).

Per-core phases (Tile framework, all loops fully unrolled):
  P1 features: broadcast times/meas to partition strips with small PE
     matmuls; sin+cos in one ACT Sin op (cos = sin(x+pi/2), per-partition
     scale/bias); one-hot via DVE is_equal against a per-partition iota.
  P2 z: per 512-col chunk, one PSUM tile accumulates 4 segments' logits into
     partition strips 32a..32a+3 using zero-padded copies of M1; one DVE copy
     moves it into z_quad [128, 4608] (partition = (seg-in-quad, head)).
  P3 softmax: reduce_max / Exp(bias=-max) / reduce_sum / reciprocal, all
     per-partition ops on [128, 4608].
  P4 phi + weighted segment-sum: 48->128->128->128->128 relu MLP in bf16
     (moving dim 512); the last layer swaps matmul operands so its output is
     row-major; attention rows are transposed per 128-tile by a selector
     matmul; PSUM accumulates attn^T . enc per segment.
  P5 rho MLP on the [8, 512] aggregate; sigmoid as 0.5*tanh(x/2)+0.5 to stay
     inside the exp/tanh ACT table set.
"""

import math

import numpy as np
import ml_dtypes

import concourse.bass as bass
import concourse.tile as tile
from concourse import bacc, mybir
from concourse.bass_utils import run_bass_kernel_spmd

F32 = mybir.dt.float32
F32R = mybir.dt.float32r
BF16 = mybir.dt.bfloat16
AF = mybir.ActivationFunctionType
ALU = mybir.AluOpType
NPBF16 = ml_dtypes.bfloat16

NCORES = 8
B, T = 64, 4096
SEG = 8                 # segments per core
SEGLEN = 4608           # 9*512 cols per segment (4096 time + 1 demo + 511 pad)
CH = 512
NCH = SEGLEN // CH
XCOLS = SEG * SEGLEN
D_IN = 48
N_MOD = 37
N_POS = 10
HEADS, DOT = 4, 64
NEG_BIG = -1e30

_CACHE = {}


def _build(zero_b1: bool, zero_b3: bool):
    nc = bacc.Bacc(
        "TRN2",
        target_bir_lowering=False,
        debug=False,
        enable_asserts=False,
        num_devices=NCORES,
    )

    def din(name, shape, dt):
        return nc.dram_tensor(name, list(shape), dt, kind="ExternalInput").ap()

    io = {}
    # per-core data
    io["t_in"] = din("t_in", (SEG, T), F32R)
    io["m_in"] = din("m_in", (SEG, T), F32R)
    io["v_in"] = din("v_in", (SEG, T), BF16)
    io["d_in"] = din("d_in", (8, SEG), F32)      # demo, transposed [feat, patient]
    # constants / weights (replicated across cores)
    io["sel80"] = din("sel80", (SEG, 80), F32R)
    io["selm0"] = din("selm0", (SEG, 111), F32R)
    io["selm1"] = din("selm1", (SEG, 111), F32R)
    io["selm2"] = din("selm2", (SEG, 74), F32R)
    io["sc80"] = din("sc80", (80, 1), F32)
    io["bi80"] = din("bi80", (80, 1), F32)
    io["npi80"] = din("npi80", (80, 1), F32)
    io["iota111"] = din("iota111", (111, 1), F32)
    io["iota74"] = din("iota74", (74, 1), F32)
    io["id4f"] = din("id4f", (4, 4), F32)
    io["m1s"] = din("m1s", (D_IN, 512), BF16)    # M1 at strip cols, per a
    io["asel"] = din("asel", (128, 16), BF16)    # attn transpose selectors
    io["w0"] = din("w0", (D_IN, 128), BF16)
    io["w1"] = din("w1", (128, 128), BF16)
    io["w2"] = din("w2", (128, 128), BF16)
    io["w3"] = din("w3", (128, 128), BF16)
    for i in range(4):
        io[f"pb{i}"] = din(f"pb{i}", (128, 1), F32)
    io["b3bc"] = din("b3bc", (128, 512), F32)    # phi_b3 broadcast (row-major)
    io["dw1"] = din("dw1", (8, 128), F32)
    io["db1"] = din("db1", (128, 1), F32)
    io["dw2"] = din("dw2", (128, D_IN), F32)
    io["db2s"] = din("db2s", (D_IN, 1), F32)   # sign-flipped rows 0..9
    io["dsc"] = din("dsc", (D_IN, 1), F32)     # -1 rows 0..9, else +1
    io["rw0"] = din("rw0", (512, 128), F32)
    io["rw1"] = din("rw1", (128, 128), F32)
    io["rw2"] = din("rw2", (128, 128), F32)
    io["rw3"] = din("rw3", (128, 1), F32)
    for i in range(3):
        io[f"rb{i}"] = din(f"rb{i}", (128, 1), F32)
    io["rb3h"] = din("rb3h", (1, 1), F32)

    io["out"] = nc.dram_tensor("out", [1, SEG], F32, kind="ExternalOutput").ap()

    with tile.TileContext(nc) as tc:
        _emit(tc, io, zero_b1, zero_b3)

    nc.compile()
    return nc


def _emit(tc, io, zero_b1, zero_b3):
    nc = tc.nc
    sync = nc.sync
    act = nc.scalar
    dve = nc.vector
    pe = nc.tensor

    with tc.tile_pool(name="const", bufs=1) as cp:
        x_T = cp.tile([D_IN, XCOLS], BF16, tag="x_T")
        z_q = [cp.tile([128, SEGLEN], F32, tag=f"z_q{q}", name=f"z_q{q}")
               for q in range(2)]
        e_q = [cp.tile([128, SEGLEN], BF16, tag=f"e_q{q}", name=f"e_q{q}")
               for q in range(2)]

        def load(name):
            src = io[name]
            t = cp.tile(list(src.shape), src.dtype, name=name + "_sb",
                        tag=name + "_sb")
            sync.dma_start(t, src)
            return t

        sel80 = load("sel80")
        selm = [load("selm0"), load("selm1"), load("selm2")]
        sc80 = load("sc80")
        bi80 = load("bi80")
        npi80 = load("npi80")
        iota111 = load("iota111")
        iota74 = load("iota74")
        id4f = load("id4f")
        m1s = load("m1s")
        asel = load("asel")
        w0 = load("w0")
        w1 = load("w1")
        w2 = load("w2")
        w3 = load("w3")
        pb = [load(f"pb{i}") for i in range(4)]
        b3bc = None if zero_b3 else load("b3bc")
        dw1 = load("dw1")
        db1 = load("db1")
        dw2 = load("dw2")
        db2s = load("db2s")
        dsc = load("dsc")
        rw1 = load("rw1")
        rw2 = load("rw2")
        rw3 = load("rw3")
        rb = [load(f"rb{i}") for i in range(3)]
        rb3h = load("rb3h")
        d_sb = load("d_in")
        rw0 = cp.tile([128, 512], F32, tag="rw0_sb")
        for h in range(4):
            sync.dma_start(rw0[:, h * 128:(h + 1) * 128],
                           io["rw0"][h * 128:(h + 1) * 128, :])

        # ---- demo encoder: demo_enc = relu(demo@W1+b1)@W2+b2, feat-major ----
        with tc.tile_pool(name="dps", bufs=1, space="PSUM") as dps:
            h1p = dps.tile([128, SEG], F32, tag="dh1p")
            pe.matmul(h1p, dw1, d_sb, start=True, stop=True)
            dh1 = cp.tile([128, SEG], F32, tag="dh1")
            act.activation(dh1, h1p, AF.Relu, bias=db1)
            dep = dps.tile([D_IN, SEG], F32, tag="dep")
            pe.matmul(dep, dw2, dh1, start=True, stop=True)
            # rows 0..9 sign-flipped: x rows 0..9 hold -tt (see Sin below)
            denc = cp.tile([D_IN, SEG], BF16, tag="denc")
            act.activation(denc, dep, AF.Identity, bias=db2s, scale=dsc)
        # demo-encoding -> col 4096 of each segment block
        x_demo = x_T.rearrange("p (s c) -> p s c", s=SEG)[:, :, T]
        sync.dma_start(x_demo, denc)

        # values -> x_T row 10 ; zero the pad cols
        for s in range(SEG):
            o = s * SEGLEN
            sync.dma_start(x_T[10:11, o:o + T], io["v_in"][s:s + 1, :])
            dve.memset(x_T[:, o + T + 1:o + SEGLEN], 0.0)

        # ---- P1: features ----
        with tc.tile_pool(name="fstage", bufs=3) as fsp, \
             tc.tile_pool(name="fpsum", bufs=3, space="PSUM") as fpp:
            for c in range(8):
                cs = c * CH
                stage_t = fsp.tile([SEG, CH], F32R, tag="stage_t")
                sync.dma_start(stage_t, io["t_in"][:, cs:cs + CH])
                stage_m = fsp.tile([SEG, CH], F32R, tag="stage_m")
                sync.dma_start(stage_m, io["m_in"][:, cs:cs + CH])

                bct = fpp.tile([80, CH], F32, tag="bct")
                pe.matmul(bct, sel80, stage_t, start=True, stop=True)
                # theta = t/ts (+pi/2 for cos rows); ACT Sin only covers
                # [-pi, pi].  Range-reduce with an int cast (works for both
                # truncating and rounding casts): y2 = theta/2pi + 0.5,
                # rr = y2 - int(y2), rr += (rr<0), sin(2pi*rr - pi) =
                # -sin(theta); the -1 is folded into W0/M1/demo_enc rows 0..9.
                y2 = fsp.tile([80, CH], F32, tag="y2")
                act.activation(y2, bct, AF.Identity, bias=bi80, scale=sc80)
                ki = fsp.tile([80, CH], mybir.dt.int32, tag="ki")
                dve.tensor_copy(ki, y2)
                kf = fsp.tile([80, CH], F32, tag="kf")
                dve.tensor_copy(kf, ki)
                rr = fsp.tile([80, CH], F32, tag="rr")
                dve.tensor_tensor(rr, y2, kf, ALU.subtract)
                lt = fsp.tile([80, CH], F32, tag="lt")
                dve.tensor_scalar(lt, rr, 0.0, None, ALU.is_lt)
                sm = fsp.tile([80, CH], F32, tag="sm")
                dve.tensor_tensor(sm, rr, lt, ALU.add)
                sin_st = fsp.tile([80, CH], BF16, tag="sin_st")
                act.activation(sin_st, sm, AF.Sin, bias=npi80,
                               scale=2.0 * math.pi)
                for s in range(SEG):
                    sync.dma_start(
                        x_T[0:N_POS, s * SEGLEN + cs:s * SEGLEN + cs + CH],
                        sin_st[s * N_POS:(s + 1) * N_POS, :])

                for g in range(3):
                    nseg = 2 if g == 2 else 3
                    iota = iota74 if g == 2 else iota111
                    s0 = g * 3
                    nr = nseg * N_MOD
                    bcm = fpp.tile([111, CH], F32, tag="bcm")
                    pe.matmul(bcm[0:nr, :], selm[g][:, 0:nr], stage_m,
                              start=True, stop=True)
                    oh_st = fsp.tile([111, CH], BF16, tag="oh_st")
                    dve.tensor_scalar(oh_st[0:nr, :], bcm[0:nr, :],
                                      iota, None, ALU.is_equal)
                    for si in range(nseg):
                        s = s0 + si
                        sync.dma_start(
                            x_T[11:48, s * SEGLEN + cs:s * SEGLEN + cs + CH],
                            oh_st[si * N_MOD:(si + 1) * N_MOD, :])

        # ---- P2: z logits ----
        with tc.tile_pool(name="zpsum", bufs=2, space="PSUM") as zpp:
            for q in range(2):
                for c in range(NCH):
                    zp = zpp.tile([128, CH], F32, tag="zp")
                    for a in range(4):
                        o = (4 * q + a) * SEGLEN + c * CH
                        pe.matmul(zp, m1s[:, a * 128:(a + 1) * 128],
                                  x_T[:, o:o + CH],
                                  start=(a == 0), stop=(a == 3))
                    dve.tensor_copy(z_q[q][:, c * CH:(c + 1) * CH], zp)

        # ---- P3: segment softmax pieces ----
        inv_s = []
        for q in range(2):
            dve.memset(z_q[q][:, T + 1:SEGLEN], NEG_BIG)
            mx = cp.tile([128, 1], F32, tag=f"mx{q}", name=f"mx{q}")
            dve.reduce_max(mx, z_q[q], axis=mybir.AxisListType.X)
            negm = cp.tile([128, 1], F32, tag=f"negm{q}", name=f"negm{q}")
            dve.tensor_scalar_mul(negm, mx, -1.0)
            act.activation(e_q[q], z_q[q], AF.Exp, bias=negm)
            ssum = cp.tile([128, 1], F32, tag=f"ssum{q}", name=f"ssum{q}")
            dve.reduce_sum(ssum, e_q[q], axis=mybir.AxisListType.X)
            iv = cp.tile([128, 1], F32, tag=f"invs{q}", name=f"invs{q}")
            dve.reciprocal(iv, ssum)
            inv_s.append(iv)
        # per-segment 1/sum at partitions 0..3
        inv_seg = []
        for s in range(SEG):
            q, a = divmod(s, 4)
            ivs = cp.tile([HEADS, 1], F32, tag=f"ivseg{s}", name=f"ivseg{s}")
            sync.dma_start(ivs, inv_s[q][32 * a:32 * a + HEADS, :])
            inv_seg.append(ivs)

        # ---- P4: phi MLP + weighted segment sum ----
        agg_sb = []
        with tc.tile_pool(name="mlp", bufs=2, space="PSUM") as mpp, \
             tc.tile_pool(name="encp", bufs=2, space="PSUM") as epp, \
             tc.tile_pool(name="atps", bufs=2, space="PSUM") as app, \
             tc.tile_pool(name="aggp", bufs=2, space="PSUM") as gpp, \
             tc.tile_pool(name="work", bufs=3) as wp:
            for s in range(SEG):
                q, a = divmod(s, 4)
                agg = gpp.tile([HEADS, 128], F32, tag="agg")
                for c in range(NCH):
                    o = s * SEGLEN + c * CH
                    h0p = mpp.tile([128, CH], F32, tag="mlp")
                    pe.matmul(h0p, w0, x_T[:, o:o + CH], start=True, stop=True)
                    h0 = wp.tile([128, CH], BF16, tag="h0")
                    act.activation(h0, h0p, AF.Relu, bias=pb[0])
                    h1p = mpp.tile([128, CH], F32, tag="mlp")
                    pe.matmul(h1p, w1, h0, start=True, stop=True)
                    h1 = wp.tile([128, CH], BF16, tag="h1")
                    if zero_b1:
                        dve.tensor_scalar_max(h1, h1p, 0.0)
                    else:
                        dve.tensor_scalar(h1, h1p, pb[1], 0.0, ALU.add, ALU.max)
                    h2p = mpp.tile([128, CH], F32, tag="mlp")
                    pe.matmul(h2p, w2, h1, start=True, stop=True)
                    h2 = wp.tile([128, CH], BF16, tag="h2")
                    act.activation(h2, h2p, AF.Relu, bias=pb[2])
                    encp = epp.tile([128, CH], F32, tag="enc")
                    for t in range(4):
                        pe.matmul(encp[:, t * 128:(t + 1) * 128],
                                  h2[:, t * 128:(t + 1) * 128], w3,
                                  start=True, stop=True)
                    enc = wp.tile([128, CH], BF16, tag="enc")
                    if zero_b3:
                        dve.tensor_scalar_max(enc, encp, 0.0)
                    else:
                        dve.tensor_tensor(enc, encp, b3bc, ALU.add)
                        dve.tensor_scalar_max(enc, enc, 0.0)
                    atp = app.tile([128, 16], F32, tag="atp")
                    for t in range(4):
                        ec = c * CH + t * 128
                        pe.matmul(atp[:, t * 4:(t + 1) * 4],
                                  e_q[q][:, ec:ec + 128],
                                  asel[:, a * 4:(a + 1) * 4],
                                  start=True, stop=True)
                    attn = wp.tile([128, 16], BF16, tag="attn")
                    dve.tensor_copy(attn, atp)
                    for t in range(4):
                        pe.matmul(agg, attn[:, t * 4:(t + 1) * 4],
                                  enc[:, t * 128:(t + 1) * 128],
                                  start=(c == 0 and t == 0),
                                  stop=(c == NCH - 1 and t == 3),
                                  skip_group_check=True)
                asb = cp.tile([HEADS, 128], F32, tag=f"aggsb{s}",
                              name=f"aggsb{s}")
                act.activation(asb, agg, AF.Copy, scale=inv_seg[s])
                agg_sb.append(asb)

        # ---- P5: rho MLP ----
        with tc.tile_pool(name="rps", bufs=1, space="PSUM") as rps, \
             tc.tile_pool(name="rwork", bufs=1) as rwp:
            rtp = rps.tile([128, 32], F32, tag="rtp")
            for s in range(SEG):
                pe.matmul(rtp[:, s * 4:(s + 1) * 4], agg_sb[s], id4f,
                          start=True, stop=True, skip_group_check=True)
            rho_in = rwp.tile([128, 32], F32, tag="rho_in")
            dve.tensor_copy(
                rho_in.rearrange("p (h s) -> p h s", h=4),
                rtp.rearrange("p (s h) -> p h s", s=SEG))
            r1p = rps.tile([128, SEG], F32, tag="r1p")
            for h in range(4):
                pe.matmul(r1p, rw0[:, h * 128:(h + 1) * 128],
                          rho_in[:, h * SEG:(h + 1) * SEG],
                          start=(h == 0), stop=(h == 3))
            r1 = rwp.tile([128, SEG], F32, tag="r1")
            act.activation(r1, r1p, AF.Relu, bias=rb[0])
            r2p = rps.tile([128, SEG], F32, tag="r2p")
            pe.matmul(r2p, rw1, r1, start=True, stop=True)
            r2 = rwp.tile([128, SEG], F32, tag="r2")
            act.activation(r2, r2p, AF.Relu, bias=rb[1])
            r3p = rps.tile([128, SEG], F32, tag="r3p")
            pe.matmul(r3p, rw2, r2, start=True, stop=True)
            r3 = rwp.tile([128, SEG], F32, tag="r3")
            act.activation(r3, r3p, AF.Relu, bias=rb[2])
            otp = rps.tile([1, SEG], F32, tag="otp")
            pe.matmul(otp, rw3, r3, start=True, stop=True)
            th = rwp.tile([1, SEG], F32, tag="th")
            act.activation(th, otp, AF.Tanh, bias=rb3h, scale=0.5)
            osb = rwp.tile([1, SEG], F32, tag="osb")
            act.activation(osb, th, AF.Copy, bias=0.5, scale=0.5)
            sync.dma_start(io["out"], osb)


def host_prep(inputs):
    """Host-side input prep: sharding, dtype casts, O(weights) constant folds."""
    f32 = np.float32
    times = np.asarray(inputs["times"], f32).reshape(B, T)
    values = np.asarray(inputs["values"], f32).reshape(B, T)
    meas = np.asarray(inputs["measurements"])
    demo = np.asarray(inputs["demo"], f32)
    timescales = np.asarray(inputs["timescales"], f32)
    seg_ids = np.asarray(inputs["segment_ids"])
    expect = np.repeat(np.arange(B, dtype=seg_ids.dtype), T + 1)
    assert seg_ids.shape == expect.shape and np.array_equal(seg_ids, expect), \
        "kernel assumes full-length segments (repeat(arange(B), T+1))"

    W_k = np.asarray(inputs["W_k"], f32)
    W_q = np.asarray(inputs["W_q"], f32)
    M1 = np.einsum("ihd,hd->ih", W_k[:D_IN].reshape(D_IN, HEADS, DOT),
                   W_q) / np.sqrt(f32(DOT))
    # x rows 0..9 hold -sin/-cos on device; flip matching weight rows
    sgn = np.ones((D_IN, 1), f32)
    sgn[:N_POS] = -1.0
    M1 = M1 * sgn
    m1s = np.zeros((D_IN, 512), f32)
    for a in range(4):
        for h in range(HEADS):
            m1s[:, a * 128 + 32 * a + h] = M1[:, h]
    asel = np.zeros((128, 16), f32)
    for a in range(4):
        for h in range(HEADS):
            asel[32 * a + h, a * 4 + h] = 1.0
    sel80 = np.zeros((SEG, 80), f32)
    for s in range(SEG):
        sel80[s, s * N_POS:(s + 1) * N_POS] = 1.0
    selm0 = np.zeros((SEG, 111), f32)
    selm1 = np.zeros((SEG, 111), f32)
    selm2 = np.zeros((SEG, 74), f32)
    for si in range(3):
        selm0[si, si * N_MOD:(si + 1) * N_MOD] = 1.0
        selm1[3 + si, si * N_MOD:(si + 1) * N_MOD] = 1.0
    for si in range(2):
        selm2[6 + si, si * N_MOD:(si + 1) * N_MOD] = 1.0
    # y2 = theta/(2pi) with theta = t/ts (+pi/2 on cos rows);
    # sin(2pi*frac(y2) - pi) = -sin(theta) (resp. -cos on the +0.25 rows)
    inv_ts2pi = (1.0 / (2.0 * math.pi * timescales)).astype(f32)
    sc80 = np.zeros((80, 1), f32)
    bi80 = np.zeros((80, 1), f32)
    for s in range(SEG):
        sc80[s * N_POS:s * N_POS + 5, 0] = inv_ts2pi
        sc80[s * N_POS + 5:s * N_POS + 10, 0] = inv_ts2pi
        bi80[s * N_POS + 5:s * N_POS + 10, 0] = 0.25
    iota111 = (np.arange(111) % N_MOD).astype(f32).reshape(111, 1)
    iota74 = (np.arange(74) % N_MOD).astype(f32).reshape(74, 1)

    phi_b1 = np.asarray(inputs["phi_b1"], f32)
    phi_b3 = np.asarray(inputs["phi_b3"], f32)
    zero_b1 = bool(np.all(phi_b1 == 0))
    zero_b3 = bool(np.all(phi_b3 == 0))

    consts = {
        "sel80": sel80, "selm0": selm0, "selm1": selm1, "selm2": selm2,
        "sc80": sc80, "bi80": bi80,
        "npi80": np.full((80, 1), -math.pi, f32),
        "iota111": iota111, "iota74": iota74,
        "id4f": np.eye(4, dtype=f32),
        "m1s": m1s.astype(NPBF16), "asel": asel.astype(NPBF16),
        "w0": (np.asarray(inputs["phi_W0"], f32) * sgn).astype(NPBF16),
        "w1": np.asarray(inputs["phi_W1"], f32).astype(NPBF16),
        "w2": np.asarray(inputs["phi_W2"], f32).astype(NPBF16),
        "w3": np.asarray(inputs["phi_W3"], f32).astype(NPBF16),
        "pb0": np.asarray(inputs["phi_b0"], f32).reshape(128, 1),
        "pb1": phi_b1.reshape(128, 1),
        "pb2": np.asarray(inputs["phi_b2"], f32).reshape(128, 1),
        "pb3": phi_b3.reshape(128, 1),
        "b3bc": np.tile(phi_b3.reshape(1, 128), (128, 4)).astype(f32),
        "dw1": np.asarray(inputs["demo_W1"], f32),
        "db1": np.asarray(inputs["demo_b1"], f32).reshape(128, 1),
        "dw2": np.asarray(inputs["demo_W2"], f32),
        "db2s": (np.asarray(inputs["demo_b2"], f32).reshape(D_IN, 1) * sgn),
        "dsc": sgn.copy(),
        "rw0": np.asarray(inputs["rho_W0"], f32),
        "rw1": np.asarray(inputs["rho_W1"], f32),
        "rw2": np.asarray(inputs["rho_W2"], f32),
        "rw3": np.asarray(inputs["rho_W3"], f32).reshape(128, 1),
        "rb0": np.asarray(inputs["rho_b0"], f32).reshape(128, 1),
        "rb1": np.asarray(inputs["rho_b1"], f32).reshape(128, 1),
        "rb2": np.asarray(inputs["rho_b2"], f32).reshape(128, 1),
        "rb3h": (0.5 * np.asarray(inputs["rho_b3"], f32)).reshape(1, 1),
    }
    in_maps = []
    for c in range(NCORES):
        lo, hi = c * SEG, (c + 1) * SEG
        m = dict(consts)
        m["t_in"] = np.ascontiguousarray(times[lo:hi])
        m["m_in"] = np.ascontiguousarray(meas[lo:hi].astype(f32))
        m["v_in"] = np.ascontiguousarray(values[lo:hi].astype(NPBF16))
        m["d_in"] = np.ascontiguousarray(demo[lo:hi].T)
        in_maps.append(m)
    return in_maps, zero_b1, zero_b3


def get_nc(zero_b1, zero_b3):
    key = (zero_b1, zero_b3)
    if key not in _CACHE:
        _CACHE[key] = _build(zero_b1, zero_b3)
    return _CACHE[key]


def kernel(**inputs):
    in_maps, zero_b1, zero_b3 = host_prep(inputs)
    nc = get_nc(zero_b1, zero_b3)
    res = run_bass_kernel_spmd(nc, in_maps, core_ids=list(range(NCORES)))
    out = np.empty((B, 1), np.float32)
    for c in range(NCORES):
        out[c * SEG:(c + 1) * SEG, 0] = np.asarray(res.results[c]["out"])[0]
    return out
